# revision 9
# baseline (speedup 1.0000x reference)
"""Trainium2 Bass kernel for nn_AttentionNeNode (8-core SPMD).

Math being computed (see problem reference):
    sel  = inputs[:, in_idxs]            # [R, L] column gather
    qkv  = sel @ weights                 # [R, 3] -> q, k, v columns
    out  = sigmoid(softmax(q[-1] * k.T) @ v)   # only the LAST row's attention matters

Key transformations:
  1. Column gather + matmul == dense matmul with scattered weights:
         sel @ weights == inputs @ W_dense,
     where W_dense[f] = sum of weights[l] over l with in_idxs[l] == f.
     This turns random column access into a dense streaming read of `inputs`.
  2. Only row R-1's attention is needed, so each core computes k, v for its
     block of rows plus flash-softmax partial stats (max, sum_exp, sum_exp*v)
     per 512-row slice; host combines the 16 stat triples (the "unshard").
  3. `inputs` is pre-transposed/tiled on host so the contraction dim (F) lands
     on SBUF partitions and DMA descriptors are large contiguous runs.
  4. k and v come out of ONE m=2 fp32r matmul per chunk (rhs streamed once).
     v (psum partition 1) is moved to partition 0 via a tiny SBUF->SBUF DMA
     that overlaps the k-side softmax stats.
"""

import sys

if "/opt/trn_rl_repo" not in sys.path:
    sys.path.insert(0, "/opt/trn_rl_repo")

import numpy as np

import concourse.bacc as bacc
import concourse.tile as tile
from concourse import mybir
from concourse.bass_utils import run_bass_kernel_spmd

R, F = 8192, 4096
NCORES = 8
RB = R // NCORES            # 1024 rows per core
NCHUNK = F // 128           # 32 contraction chunks of 128
# row slices per core: (start_col, rows). Descending sizes: the LAST slice's
# softmax-stats chain is serially exposed after the final DMA, so keep it small.
SLICES = [(0, 512), (512, 384), (896, 128)]
NSLICE = len(SLICES)
CH_PER_TILE = 8             # f-chunks per DMA tile
F32 = mybir.dt.float32
F32R = mybir.dt.float32r

_NC = None


def _build_nc():
    nc = bacc.Bacc("TRN2", target_bir_lowering=False, debug=False)
    xt = nc.dram_tensor("xt", [128, NCHUNK, RB], F32R,
                        kind="ExternalInput").ap()
    wsb = nc.dram_tensor("wsb", [128, 3 * NCHUNK], F32R, kind="ExternalInput").ap()
    # last-row chunks duplicated x2: fp32r matmul needs moving free dim >= 2
    lrow = nc.dram_tensor("lrow", [128, 2 * NCHUNK], F32R,
                          kind="ExternalInput").ap()
    out = nc.dram_tensor("out", [1, 12], F32, kind="ExternalOutput").ap()

    AF = mybir.ActivationFunctionType
    ALU = mybir.AluOpType
    AX = mybir.AxisListType

    with tile.TileContext(nc) as tc:
        with tc.tile_pool(name="consts", bufs=1) as consts, \
             tc.tile_pool(name="xtiles", bufs=3) as xtiles, \
             tc.tile_pool(name="ps", bufs=2, space="PSUM") as psp, \
             tc.tile_pool(name="psq", bufs=1, space="PSUM") as psqp, \
             tc.tile_pool(name="tail", bufs=2) as tailp, \
             tc.tile_pool(name="fin", bufs=1) as finp:
            # const loads go FIRST on the sync queue: they are tiny, and any
            # other placement lets the 2 MiB tile stream starve them (SDMA
            # round-robins rings at packet granularity, stalling the PE FIFO)
            w_t = consts.tile([128, 3 * NCHUNK], F32R)
            nc.sync.dma_start(out=w_t[:], in_=wsb)
            l_t = consts.tile([128, 2 * NCHUNK], F32R)
            nc.sync.dma_start(out=l_t[:], in_=lrow)

            ps_q = psqp.tile([1, 2], F32)
            qlast = finp.tile([1, 1], F32)
            outsb = finp.tile([1, 12], F32)
            nc.vector.memset(outsb[:], 0.0)

            for s, (r0, rows) in enumerate(SLICES):
                # one m=2 matmul per chunk makes [k; v] rows: stream rhs ONCE
                ps_kv = psp.tile([2, rows], F32, tag="ps_kv")
                c0 = 0
                while c0 < NCHUNK:
                    nt = min(CH_PER_TILE, NCHUNK - c0)
                    x_t = xtiles.tile([128, nt, rows], F32R, tag="x_t")
                    nc.sync.dma_start(out=x_t[:],
                                      in_=xt[:, c0:c0 + nt, r0:r0 + rows])
                    for u in range(nt):
                        c = c0 + u
                        rhs = x_t[:, u, :]
                        st, sp = (c == 0), (c == NCHUNK - 1)
                        nc.tensor.matmul(ps_kv[:], w_t[:, 3 * c + 1:3 * c + 3],
                                         rhs, start=st, stop=sp)
                        if s == 0:
                            nc.tensor.matmul(ps_q[:], w_t[:, 3 * c:3 * c + 1],
                                             l_t[:, 2 * c:2 * c + 2],
                                             start=st, stop=sp)
                    c0 += nt
                if s == 0:
                    nc.scalar.copy(out=qlast[:], in_=ps_q[:, 0:1])
                # evacuate PSUM; k stays on partition 0 (directly usable),
                # v (partition 1) is flattened down via a small SBUF->SBUF DMA
                # that overlaps the k-side stats chain below
                kv_sb = tailp.tile([2, rows], F32, tag="kv_sb")
                nc.scalar.copy(out=kv_sb[:], in_=ps_kv[:])
                v_f = tailp.tile([1, rows], F32, tag="v_f")
                nc.sync.dma_start(out=v_f[:], in_=kv_sb[1:2, :])
                # flash-softmax partial stats for this slice; logits reads k
                # straight from PSUM partition 0 so the k-side chain runs
                # concurrently with the kv copy + v flatten DMA
                logits = tailp.tile([1, rows], F32, tag="logits")
                nc.vector.tensor_scalar_mul(out=logits[:], in0=ps_kv[0:1, :],
                                            scalar1=qlast[:])
                nc.vector.tensor_reduce(out=outsb[:, 3 * s:3 * s + 1],
                                        in_=logits[:], axis=AX.X, op=ALU.max,
                                        negate=True)
                e_t = tailp.tile([1, rows], F32, tag="e_t")
                nc.scalar.activation(out=e_t[:], in_=logits[:], func=AF.Exp,
                                     bias=outsb[:, 3 * s:3 * s + 1], scale=1.0,
                                     accum_out=outsb[:, 3 * s + 1:3 * s + 2])
                scr = tailp.tile([1, rows], F32, tag="scr")
                nc.vector.tensor_mul(out=scr[:], in0=e_t[:], in1=v_f[:])
                nc.vector.reduce_sum(out=outsb[:, 3 * s + 2:3 * s + 3],
                                     in_=scr[:], axis=AX.X)

            nc.sync.dma_start(out=out, in_=outsb[:])
    nc.finalize()
    return nc


def _get_nc():
    global _NC
    if _NC is None:
        _NC = _build_nc()
    return _NC


def _prep_inputs(inputs, in_idxs, weights):
    inputs = np.ascontiguousarray(np.asarray(inputs, dtype=np.float32))
    idx = np.asarray(in_idxs).astype(np.int64)
    w = np.asarray(weights, dtype=np.float32)

    # scatter-add weights into dense [F, 3]: sel @ weights == inputs @ wd
    wd = np.zeros((F, 3), dtype=np.float32)
    np.add.at(wd, idx, w)
    # SBUF layout [128, 3*NCHUNK]: wsb[p, 3c+j] = wd[c*128+p, j]
    wsb = np.ascontiguousarray(
        wd.reshape(NCHUNK, 128, 3).transpose(1, 0, 2).reshape(128, 3 * NCHUNK))
    # last row of inputs, chunked + duplicated: lrow[p, 2c+{0,1}] = x[R-1, c*128+p]
    lrow = np.ascontiguousarray(
        np.repeat(inputs[R - 1].reshape(NCHUNK, 128).T, 2, axis=1))

    # xt[core][p, c, col] = inputs[core*RB + col, c*128 + p]
    x4 = inputs.reshape(NCORES, RB, NCHUNK, 128)
    xt_all = np.ascontiguousarray(x4.transpose(0, 3, 2, 1))

    return [{"xt": xt_all[i], "wsb": wsb, "lrow": lrow} for i in range(NCORES)]


def _combine(outs):
    # outs: [N, 12]: per slice s: (-max_logit, sum_exp, sum_exp_v) at columns
    # 3s..3s+2. Exact flash-softmax combine in fp64 on the host.
    o = np.asarray(outs, dtype=np.float64)
    trip = np.concatenate([o[:, 3 * s:3 * s + 3] for s in range(NSLICE)], axis=0)
    m = -trip[:, 0]
    s = trip[:, 1]
    w = trip[:, 2]
    mx = m.max()
    scale = np.exp(m - mx)
    val = (w * scale).sum() / (s * scale).sum()
    return np.array([[1.0 / (1.0 + np.exp(-val))]], dtype=np.float32)


def kernel(inputs, in_idxs, weights):
    nc = _get_nc()
    in_maps = _prep_inputs(inputs, in_idxs, weights)
    res = run_bass_kernel_spmd(nc, in_maps, core_ids=list(range(NCORES)))
    outs = np.stack([res.results[i]["out"][0] for i in range(NCORES)])
    return _combine(outs)


if __name__ == "__main__":
    rng = np.random.default_rng(0)
    inputs = rng.standard_normal((R, F), dtype=np.float32)
    in_idxs = rng.integers(0, F, size=2048)
    weights = rng.standard_normal((2048, 3), dtype=np.float32)
    got = kernel(inputs, in_idxs, weights)
    sel = inputs[:, in_idxs]
    qkv = sel.astype(np.float64) @ weights.astype(np.float64)
    q, k, v = qkv[:, 0], qkv[:, 1], qkv[:, 2]
    logits = q[-1] * k
    a = np.exp(logits - logits.max())
    want = a @ v / a.sum()
    want = 1.0 / (1.0 + np.exp(-want))
    print("got", got, "want", want, "relerr", abs(got[0, 0] - want) / max(abs(want), 1e-30))


# revision 10
# speedup vs baseline: 1.0296x; 1.0296x over previous
"""Trainium2 Bass kernel for nn_AttentionNeNode (8-core SPMD).

Math being computed (see problem reference):
    sel  = inputs[:, in_idxs]            # [R, L] column gather
    qkv  = sel @ weights                 # [R, 3] -> q, k, v columns
    out  = sigmoid(softmax(q[-1] * k.T) @ v)   # only the LAST row's attention matters

Key transformations:
  1. Column gather + matmul == dense matmul with scattered weights:
         sel @ weights == inputs @ W_dense,
     where W_dense[f] = sum of weights[l] over l with in_idxs[l] == f.
     This turns random column access into a dense streaming read of `inputs`.
  2. Only row R-1's attention is needed, so each core computes k, v for its
     block of rows plus flash-softmax partial stats (max, sum_exp, sum_exp*v)
     per 512-row slice; host combines the 16 stat triples (the "unshard").
  3. `inputs` is pre-transposed/tiled on host so the contraction dim (F) lands
     on SBUF partitions and DMA descriptors are large contiguous runs.
  4. k and v come out of ONE m=2 fp32r matmul per chunk (rhs streamed once).
     v (psum partition 1) is moved to partition 0 via a tiny SBUF->SBUF DMA
     that overlaps the k-side softmax stats.
"""

import sys

if "/opt/trn_rl_repo" not in sys.path:
    sys.path.insert(0, "/opt/trn_rl_repo")

import numpy as np

import concourse.bacc as bacc
import concourse.tile as tile
from concourse import mybir
from concourse.bass_utils import run_bass_kernel_spmd

R, F = 8192, 4096
NCORES = 8
RB = R // NCORES            # 1024 rows per core
NCHUNK = F // 128           # 32 contraction chunks of 128
# row slices per core: (start_col, rows). Descending sizes: the LAST slice's
# softmax-stats chain is serially exposed after the final DMA, so keep it small.
SLICES = [(0, 512), (512, 384), (896, 128)]
NSLICE = len(SLICES)
CH_PER_TILE = 8             # f-chunks per DMA tile
F32 = mybir.dt.float32
F32R = mybir.dt.float32r

_NC = None


def _build_nc():
    nc = bacc.Bacc("TRN2", target_bir_lowering=False, debug=False)
    # one tensor per row-slice, each laid out [p, c, col] so a DMA of a chunk
    # range is a single large contiguous run per partition (16/12/4 KiB)
    xts = [nc.dram_tensor(f"xt{s}", [128, NCHUNK, rows], F32R,
                          kind="ExternalInput").ap()
           for s, (_, rows) in enumerate(SLICES)]
    wsb = nc.dram_tensor("wsb", [128, 3 * NCHUNK], F32R, kind="ExternalInput").ap()
    # last-row chunks duplicated x2: fp32r matmul needs moving free dim >= 2
    lrow = nc.dram_tensor("lrow", [128, 2 * NCHUNK], F32R,
                          kind="ExternalInput").ap()
    out = nc.dram_tensor("out", [1, 12], F32, kind="ExternalOutput").ap()

    AF = mybir.ActivationFunctionType
    ALU = mybir.AluOpType
    AX = mybir.AxisListType

    with tile.TileContext(nc) as tc:
        with tc.tile_pool(name="consts", bufs=1) as consts, \
             tc.tile_pool(name="xtiles", bufs=3) as xtiles, \
             tc.tile_pool(name="ps", bufs=2, space="PSUM") as psp, \
             tc.tile_pool(name="psq", bufs=1, space="PSUM") as psqp, \
             tc.tile_pool(name="tail", bufs=2) as tailp, \
             tc.tile_pool(name="fin", bufs=1) as finp:
            # const loads go FIRST on the sync queue: they are tiny, and any
            # other placement lets the 2 MiB tile stream starve them (SDMA
            # round-robins rings at packet granularity, stalling the PE FIFO)
            w_t = consts.tile([128, 3 * NCHUNK], F32R)
            nc.sync.dma_start(out=w_t[:], in_=wsb)
            l_t = consts.tile([128, 2 * NCHUNK], F32R)
            nc.sync.dma_start(out=l_t[:], in_=lrow)

            ps_q = psqp.tile([1, 2], F32)
            qlast = finp.tile([1, 1], F32)
            outsb = finp.tile([1, 12], F32)
            nc.vector.memset(outsb[:], 0.0)

            for s, (r0, rows) in enumerate(SLICES):
                # one m=2 matmul per chunk makes [k; v] rows: stream rhs ONCE
                ps_kv = psp.tile([2, rows], F32, tag="ps_kv")
                c0 = 0
                while c0 < NCHUNK:
                    nt = min(CH_PER_TILE, NCHUNK - c0)
                    x_t = xtiles.tile([128, nt, rows], F32R, tag="x_t")
                    nc.sync.dma_start(out=x_t[:],
                                      in_=xts[s][:, c0:c0 + nt, :])
                    for u in range(nt):
                        c = c0 + u
                        rhs = x_t[:, u, :]
                        st, sp = (c == 0), (c == NCHUNK - 1)
                        nc.tensor.matmul(ps_kv[:], w_t[:, 3 * c + 1:3 * c + 3],
                                         rhs, start=st, stop=sp)
                        if s == 0:
                            nc.tensor.matmul(ps_q[:], w_t[:, 3 * c:3 * c + 1],
                                             l_t[:, 2 * c:2 * c + 2],
                                             start=st, stop=sp)
                    c0 += nt
                if s == 0:
                    nc.scalar.copy(out=qlast[:], in_=ps_q[:, 0:1])
                # evacuate PSUM; k stays on partition 0 (directly usable),
                # v (partition 1) is flattened down via a small SBUF->SBUF DMA
                # that overlaps the k-side stats chain below
                kv_sb = tailp.tile([2, rows], F32, tag="kv_sb")
                nc.scalar.copy(out=kv_sb[:], in_=ps_kv[:])
                v_f = tailp.tile([1, rows], F32, tag="v_f")
                nc.sync.dma_start(out=v_f[:], in_=kv_sb[1:2, :])
                # flash-softmax partial stats for this slice; logits reads k
                # straight from PSUM partition 0 so the k-side chain runs
                # concurrently with the kv copy + v flatten DMA
                logits = tailp.tile([1, rows], F32, tag="logits")
                nc.vector.tensor_scalar_mul(out=logits[:], in0=ps_kv[0:1, :],
                                            scalar1=qlast[:])
                nc.vector.tensor_reduce(out=outsb[:, 3 * s:3 * s + 1],
                                        in_=logits[:], axis=AX.X, op=ALU.max,
                                        negate=True)
                e_t = tailp.tile([1, rows], F32, tag="e_t")
                nc.scalar.activation(out=e_t[:], in_=logits[:], func=AF.Exp,
                                     bias=outsb[:, 3 * s:3 * s + 1], scale=1.0,
                                     accum_out=outsb[:, 3 * s + 1:3 * s + 2])
                scr = tailp.tile([1, rows], F32, tag="scr")
                nc.vector.tensor_mul(out=scr[:], in0=e_t[:], in1=v_f[:])
                nc.vector.reduce_sum(out=outsb[:, 3 * s + 2:3 * s + 3],
                                     in_=scr[:], axis=AX.X)

            nc.sync.dma_start(out=out, in_=outsb[:])
    nc.finalize()
    return nc


def _get_nc():
    global _NC
    if _NC is None:
        _NC = _build_nc()
    return _NC


def _prep_inputs(inputs, in_idxs, weights):
    inputs = np.ascontiguousarray(np.asarray(inputs, dtype=np.float32))
    idx = np.asarray(in_idxs).astype(np.int64)
    w = np.asarray(weights, dtype=np.float32)

    # scatter-add weights into dense [F, 3]: sel @ weights == inputs @ wd
    wd = np.zeros((F, 3), dtype=np.float32)
    np.add.at(wd, idx, w)
    # SBUF layout [128, 3*NCHUNK]: wsb[p, 3c+j] = wd[c*128+p, j]
    wsb = np.ascontiguousarray(
        wd.reshape(NCHUNK, 128, 3).transpose(1, 0, 2).reshape(128, 3 * NCHUNK))
    # last row of inputs, chunked + duplicated: lrow[p, 2c+{0,1}] = x[R-1, c*128+p]
    lrow = np.ascontiguousarray(
        np.repeat(inputs[R - 1].reshape(NCHUNK, 128).T, 2, axis=1))

    # xt{s}[core][p, c, col] = inputs[core*RB + r0 + col, c*128 + p]
    x4 = inputs.reshape(NCORES, RB, NCHUNK, 128).transpose(0, 3, 2, 1)
    maps = []
    for i in range(NCORES):
        m = {"wsb": wsb, "lrow": lrow}
        for s, (r0, rows) in enumerate(SLICES):
            m[f"xt{s}"] = np.ascontiguousarray(x4[i, :, :, r0:r0 + rows])
        maps.append(m)
    return maps


def _combine(outs):
    # outs: [N, 12]: per slice s: (-max_logit, sum_exp, sum_exp_v) at columns
    # 3s..3s+2. Exact flash-softmax combine in fp64 on the host.
    o = np.asarray(outs, dtype=np.float64)
    trip = np.concatenate([o[:, 3 * s:3 * s + 3] for s in range(NSLICE)], axis=0)
    m = -trip[:, 0]
    s = trip[:, 1]
    w = trip[:, 2]
    mx = m.max()
    scale = np.exp(m - mx)
    val = (w * scale).sum() / (s * scale).sum()
    return np.array([[1.0 / (1.0 + np.exp(-val))]], dtype=np.float32)


def kernel(inputs, in_idxs, weights):
    nc = _get_nc()
    in_maps = _prep_inputs(inputs, in_idxs, weights)
    res = run_bass_kernel_spmd(nc, in_maps, core_ids=list(range(NCORES)))
    outs = np.stack([res.results[i]["out"][0] for i in range(NCORES)])
    return _combine(outs)


if __name__ == "__main__":
    rng = np.random.default_rng(0)
    inputs = rng.standard_normal((R, F), dtype=np.float32)
    in_idxs = rng.integers(0, F, size=2048)
    weights = rng.standard_normal((2048, 3), dtype=np.float32)
    got = kernel(inputs, in_idxs, weights)
    sel = inputs[:, in_idxs]
    qkv = sel.astype(np.float64) @ weights.astype(np.float64)
    q, k, v = qkv[:, 0], qkv[:, 1], qkv[:, 2]
    logits = q[-1] * k
    a = np.exp(logits - logits.max())
    want = a @ v / a.sum()
    want = 1.0 / (1.0 + np.exp(-want))
    print("got", got, "want", want, "relerr", abs(got[0, 0] - want) / max(abs(want), 1e-30))
